# revision 1
# baseline (speedup 1.0000x reference)
"""Trainium2 Bass kernel for complex multi-head attention (8 NeuronCores).

Sharding: core c handles batch b = c//2 and head-group g = c%2 (8 of 16
heads, i.e. 512 of 1024 embed dims). No device collectives: each core
produces a partial out-projection (its head-group's contribution) and the
host sums the two partials per batch and adds the output bias.

Per-core dataflow (all matmuls bf16 with fp32 PSUM accumulation):
  - Q/K projections computed transposed (head-dim on partitions), written
    into stacked score operands:  R = [qr; qi], KA = [kr; ki] via ACT
    Identity(+bias) evacuations (identity is in every ACT table set, and the
    scalar engine is otherwise idle during the projection phase).
    KB = [ki; -kr] is derived from KA with cheap bf16 SBUF copies on DVE.
  - scores per (head, key-chunk, q-half): one K=128 matmul against KA gives
    attn_real^T, one against KB gives -attn_imag^T (sign irrelevant: only
    the square is used).
  - |z|^2: the PSUM->SBUF evacuation doubles as the squaring. A tunable
    share of tiles goes through ACT Square (fused, 1 elem/cycle); the rest
    through DVE copy + in-place bf16 2x multiply. The r/i halves are summed
    on GPSIMD (otherwise idle) to keep DVE for the PSUM-bound work.
  - softmax magnitudes: in-place Sqrt then Exp(scale) on ACT, batched per
    HEAD PAIR so the sqrt<->exp activation-table reloads (~2.7us each)
    halve: sqrt and exp live in different table sets.
  - V projected into VRIO = [vr | vi | ones]; the AV matmul accumulates
    [att_r | att_i | rowsum] per q-tile; the evacuation multiplies by
    1/rowsum (per-partition scalar) with the r/i pair merged in one op.
  - att is PE-transposed (head dim back onto partitions); the output
    projection writes fp32 partials, staged to SBUF on the (by then idle)
    ACT/DVE engines and DMA'd out.
  - I/O rides in 2 packed input blobs + 1 output blob per core: the axon
    relay pays ~20us/buffer/call, so buffer count dominates dispatch cost.
"""

import os
import sys

for _p in ("/opt/trn_rl_repo", "/root/.axon_site/_ro/trn_rl_repo"):
    if os.path.isdir(_p) and _p not in sys.path:
        sys.path.append(_p)

import numpy as np
import ml_dtypes

bf16 = ml_dtypes.bfloat16

P = 128
S = 1024
E = 1024
DL = 512  # local (per-core) head dims: 8 heads x 64
D = 64
HLOC = 8
SCALE = D ** -0.5

# --- tuning knobs -----------------------------------------------------------
# squares per head assigned to ACT (of 16 (kc, qh) units); rest go DVE
ACT_SQ_PER_HEAD = int(os.environ.get("K_ACT_SQ", "5"))
# r/i half sums: 'gpsimd' frees DVE for PSUM-bound work; 'dve' as fallback
ADDS_ENGINE = os.environ.get("K_ADDS", "gpsimd")
# ph1 projection evacuations: 'act' fills the idle proj window; 'dve' fallback
PH1_ENGINE = os.environ.get("K_PH1", "act")

_NC_CACHE = None


def _build():
    import concourse.tile as tile
    from concourse import bacc, mybir
    from concourse.masks import make_identity

    f32 = mybir.dt.float32
    b16 = mybir.dt.bfloat16
    Alu = mybir.AluOpType
    Act = mybir.ActivationFunctionType

    nc = bacc.Bacc("TRN2", target_bir_lowering=False, debug=False, num_devices=8)

    # All inputs ride in two packed blobs (and one packed output): the
    # axon relay pays ~20us PER BUFFER per call, so 24 separate tensors
    # would cost ~0.5ms/call in pure dispatch overhead.
    ES, EDL, DLE = E * S, E * DL, DL * E
    blob16 = nc.dram_tensor(
        "blob16", [6 * ES + 6 * EDL + 2 * DLE], b16, kind="ExternalInput").ap()
    blob32 = nc.dram_tensor(
        "blob32", [4 * P * 4 + 2 * P * DL], f32, kind="ExternalInput").ap()
    out_blob = nc.dram_tensor(
        "out", [2, S, E], f32, kind="ExternalOutput").ap()

    x_in, w_in, wo_in, bqk_in, bv_in = {}, {}, {}, {}, {}
    off = 0
    for n in ("xq_r", "xq_i", "xk_r", "xk_i", "xv_r", "xv_i"):
        x_in[n] = blob16[off:off + ES].rearrange(
            "(eo p s) -> p eo s", p=P, s=S)
        off += ES
    for n in ("wq_r", "wq_i", "wk_r", "wk_i", "wv_r", "wv_i"):
        w_in[n] = blob16[off:off + EDL].rearrange(
            "(eo p d) -> p eo d", p=P, d=DL)
        off += EDL
    for n in ("wo_r", "wo_i"):
        wo_in[n] = blob16[off:off + DLE].rearrange(
            "(dc p o) -> p dc o", p=P, o=E)
        off += DLE
    off = 0
    for n in ("bq_r", "bq_i", "bk_r", "bk_i"):
        bqk_in[n] = blob32[off:off + P * 4].rearrange("(p f) -> p f", p=P)
        off += P * 4
    for n in ("bv_r", "bv_i"):
        bv_in[n] = blob32[off:off + P * DL].rearrange("(p d) -> p d", p=P)
        off += P * DL
    out_d = {"out_r": out_blob[0], "out_i": out_blob[1]}

    with tile.TileContext(nc) as tc:
        with (
            tc.tile_pool(name="persist", bufs=1) as persist,
            tc.tile_pool(name="psum", bufs=2, space="PSUM") as psp,
            tc.tile_pool(name="recp", bufs=4) as recp,
        ):
            R_st = persist.tile([P, HLOC, S], b16, tag="R")
            KA_st = persist.tile([P, HLOC, S], b16, tag="KA")
            KB_st = persist.tile([P, HLOC, S], b16, tag="KB")
            VRIO = persist.tile([P, 8, HLOC, 130], b16, tag="VRIO")
            Att = persist.tile([P, 8, 1024], b16, tag="Att")
            AttT = persist.tile([P, 8, S], b16, tag="AttT")
            ident = persist.tile([P, P], b16, tag="ident")
            make_identity(nc, ident[:])
            nc.vector.memset(VRIO[:, :, :, 128:130], 1.0)

            bias_sb = {}
            for n, ap in bqk_in.items():
                t = persist.tile([P, 4], f32, tag=n)
                nc.sync.dma_start(t[:], ap)
                bias_sb[n] = t
            for n, ap in bv_in.items():
                t = persist.tile([P, DL], f32, tag=n)
                nc.sync.dma_start(t[:], ap)
                bias_sb[n] = t
            # negated k-biases for the "-(kr+b)" style evacuations
            for n in ("bk_r", "bk_i"):
                t = persist.tile([P, 4], f32, tag=n + "_neg")
                nc.vector.tensor_scalar(t[:], bias_sb[n][:], -1.0, None,
                                        op0=Alu.mult)
                bias_sb[n + "_neg"] = t
            wo_sb = {}
            for n, ap in wo_in.items():
                t = persist.tile([P, 4, E], b16, tag=n)
                nc.sync.dma_start(t[:], ap)
                wo_sb[n] = t

            def evac(dst, src, bias_ap, sc):
                """dst = sc * src + sc * bias  (bias_ap pre-scaled for sc=-1)."""
                if PH1_ENGINE == "act":
                    nc.scalar.activation(dst, src, Act.Identity,
                                         bias=bias_ap, scale=float(sc))
                else:
                    if sc == 1.0:
                        nc.vector.tensor_scalar(dst, src, bias_ap, None,
                                                op0=Alu.add)
                    else:
                        nc.vector.tensor_scalar(dst, src, bias_ap, sc,
                                                op0=Alu.add, op1=Alu.mult)

            # ---- Phase 1: Q/K projections (transposed out).
            # evac entries: (dest_tile, dest_base, bias_name, scale)
            # R = [qr; qi], KA = [kr; ki]; KB = [ki; -kr] derived after.
            qk_projs = [
                ("xq_r", "wq_r", R_st, 0, "bq_r", 1.0),
                ("xq_i", "wq_i", R_st, 64, "bq_i", 1.0),
                ("xk_r", "wk_r", KA_st, 0, "bk_r", 1.0),
                ("xk_i", "wk_i", KA_st, 64, "bk_i", 1.0),
            ]

            with tc.tile_pool(name="streamA", bufs=2) as stream:
                for xn, wn, dest, base, bn, sc in qk_projs:
                    x_sb = stream.tile([P, 8, S], b16, tag="x")
                    nc.sync.dma_start(x_sb[:], x_in[xn])
                    w_sb = stream.tile([P, 8, DL], b16, tag="w")
                    nc.sync.dma_start(w_sb[:], w_in[wn])
                    for dt in range(4):
                        ps = psp.tile([P, S], f32, tag="big")
                        for eo in range(8):
                            for nh in range(2):
                                nc.tensor.matmul(
                                    ps[:, nh * 512:(nh + 1) * 512],
                                    w_sb[:, eo, dt * P:(dt + 1) * P],
                                    x_sb[:, eo, nh * 512:(nh + 1) * 512],
                                    start=(eo == 0), stop=(eo == 7))
                        for half in range(2):
                            h = 2 * dt + half
                            src = ps[64 * half:64 * half + 64, :]
                            b_ap = bias_sb[bn][64 * half:64 * half + 64,
                                               dt:dt + 1]
                            evac(dest[base:base + 64, h, :], src, b_ap, sc)
                            if xn == "xk_i":
                                # KB[h] = [ki; -kr] from KA[h] (bf16 SBUF)
                                nc.vector.tensor_copy(
                                    KB_st[0:64, h, :], KA_st[64:128, h, :])
                                nc.vector.tensor_scalar(
                                    KB_st[64:128, h, :], KA_st[0:64, h, :],
                                    -1.0, None, op0=Alu.mult)

            # ---- Phases 2+3 interleaved: scores/softmax per head pair, V
            # projections slotted between pair 0's scores and pair 1.
            with (
                tc.tile_pool(name="sqp", bufs=4) as sqp,
                tc.tile_pool(name="s2p", bufs=2) as s2p,
            ):
                s2_tiles = {}

                def scores_and_mag(h):
                    """Issue score matmuls + squared-magnitude for head h."""
                    for kc in range(8):
                        s2 = s2p.tile([P, S], b16, tag=f"s2_{kc}")
                        s2_tiles[(h, kc)] = s2
                        for qh in range(2):
                            ps = psp.tile([P, S], f32, tag="big")
                            nc.tensor.matmul(
                                ps[:, 0:512],
                                KA_st[:, h, kc * P:(kc + 1) * P],
                                R_st[:, h, qh * 512:(qh + 1) * 512],
                                start=True, stop=True)
                            nc.tensor.matmul(
                                ps[:, 512:1024],
                                KB_st[:, h, kc * P:(kc + 1) * P],
                                R_st[:, h, qh * 512:(qh + 1) * 512],
                                start=True, stop=True)
                            sq = sqp.tile([P, S], b16, tag="sq")
                            if kc * 2 + qh < ACT_SQ_PER_HEAD:
                                nc.scalar.activation(sq[:], ps[:], Act.Square)
                            else:
                                nc.vector.tensor_copy(sq[:], ps[:])
                                nc.vector.tensor_tensor(
                                    sq[:], sq[:], sq[:], Alu.mult)
                            dst = s2[:, qh * 512:(qh + 1) * 512]
                            if ADDS_ENGINE == "gpsimd":
                                nc.gpsimd.tensor_tensor(
                                    dst, sq[:, 0:512], sq[:, 512:1024],
                                    Alu.add)
                            else:
                                nc.vector.tensor_tensor(
                                    dst, sq[:, 0:512], sq[:, 512:1024],
                                    Alu.add)

                def sqrt_exp(pair):
                    """Batched per pair: all sqrts, then all exps.

                    sqrt and exp live in different ACT table sets (~2.7us per
                    switch), and the Tile scheduler otherwise interleaves them
                    as s2 tiles trickle in (one reload per instruction!). The
                    zero-valued gate bias makes every sqrt of the pair depend
                    on the pair's LAST add (engine queues complete in order),
                    so all sqrts become ready at once and priority order then
                    yields [sqrt x16][exp x16] = 2 table loads per pair.
                    """
                    for h in pair:
                        for kc in range(8):
                            t = s2_tiles[(h, kc)][:]
                            nc.scalar.activation(t, t, Act.Sqrt)
                    # gate reads the last sqrt's tile: ACT completes in order,
                    # so this fires once every sqrt of the pair is done
                    gate = recp.tile([P, 1], f32, tag="gate")
                    nc.vector.tensor_scalar(
                        gate[:], s2_tiles[(pair[1], 7)][:, 0:1], 0.0, None,
                        op0=Alu.mult)
                    for h in pair:
                        for kc in range(8):
                            t = s2_tiles[(h, kc)][:]
                            nc.scalar.activation(t, t, Act.Exp, bias=gate[:],
                                                 scale=float(SCALE))

                def att_v(h):
                    """AV matmuls + normalization into Att for head h."""
                    for qt in range(8):
                        psa = psp.tile([P, 130], f32, tag="small")
                        for kc in range(8):
                            nc.tensor.matmul(
                                psa[:, 0:129],
                                s2_tiles[(h, kc)][:, qt * P:(qt + 1) * P],
                                VRIO[:, kc, h, 0:129],
                                start=(kc == 0), stop=(kc == 7))
                        rec = recp.tile([P, 1], f32, tag="rec")
                        nc.vector.reciprocal(rec[:], psa[:, 128:129])
                        # Att free layout: ri*512 + h*64 + d; both ri chunks
                        # in one op via a [P, 2, 64] view (stride 512).
                        nc.vector.tensor_scalar(
                            Att[:, qt, :].rearrange(
                                "p (ri q) -> p ri q", ri=2)[:, :, h * D:(h + 1) * D],
                            psa[:, 0:128].rearrange("p (ri d) -> p ri d", ri=2),
                            rec[:], None, op0=Alu.mult)

                def transpose_pair(p):
                    """PE-transpose the two chunks (real+imag) of pair p."""
                    for ch in (p, p + 4):
                        for qt in range(8):
                            tp = psp.tile([P, P], b16, tag="small")
                            nc.tensor.transpose(
                                tp[:], Att[:, qt, ch * P:(ch + 1) * P], ident[:])
                            nc.vector.tensor_copy(
                                AttT[:, ch, qt * P:(qt + 1) * P], tp[:])

                # pair 0 scores start as soon as the projections finish
                scores_and_mag(0)
                scores_and_mag(1)
                sqrt_exp((0, 1))

                # V projections (PE work that does not touch ACT/DVE hot path)
                with tc.tile_pool(name="streamB", bufs=2) as streamv:
                    for xn, wn, bn, ri in (("xv_r", "wv_r", "bv_r", 0),
                                           ("xv_i", "wv_i", "bv_i", 1)):
                        x_sb = streamv.tile([P, 8, S], b16, tag="xv")
                        nc.sync.dma_start(x_sb[:], x_in[xn])
                        w_sb = streamv.tile([P, 8, DL], b16, tag="wv")
                        nc.sync.dma_start(w_sb[:], w_in[wn])
                        for st in range(8):
                            ps = psp.tile([P, DL], f32, tag="med")
                            for eo in range(8):
                                nc.tensor.matmul(
                                    ps[:], x_sb[:, eo, st * P:(st + 1) * P],
                                    w_sb[:, eo, :],
                                    start=(eo == 0), stop=(eo == 7))
                            nc.vector.tensor_tensor(
                                VRIO[:, st, :, ri * D:(ri + 1) * D],
                                ps[:].rearrange("p (h d) -> p h d", h=HLOC),
                                bias_sb[bn][:].rearrange(
                                    "p (h d) -> p h d", h=HLOC),
                                Alu.add)

                    scores_and_mag(2)
                    scores_and_mag(3)
                    sqrt_exp((2, 3))

                att_v(0)
                att_v(1)
                transpose_pair(0)

                scores_and_mag(4)
                scores_and_mag(5)
                sqrt_exp((4, 5))
                att_v(2)
                att_v(3)
                transpose_pair(1)

                scores_and_mag(6)
                scores_and_mag(7)
                sqrt_exp((6, 7))
                att_v(4)
                att_v(5)
                transpose_pair(2)

                att_v(6)
                att_v(7)
                transpose_pair(3)

                # ---- Phase 5: output projections. DMA cannot read
                # PSUM, so stage through SBUF; the staging copies alternate
                # ACT/DVE (both idle during this tail).
                with tc.tile_pool(name="fin", bufs=2) as fin:
                    for ri, (wn, on) in enumerate(
                            [("wo_r", "out_r"), ("wo_i", "out_i")]):
                        for st in range(8):
                            ps = psp.tile([P, S], f32, tag="big")
                            for oh in range(2):
                                for dc in range(4):
                                    nc.tensor.matmul(
                                        ps[:, oh * 512:(oh + 1) * 512],
                                        AttT[:, ri * 4 + dc,
                                             st * P:(st + 1) * P],
                                        wo_sb[wn][:, dc,
                                                  oh * 512:(oh + 1) * 512],
                                        start=(dc == 0), stop=(dc == 3))
                            ob = fin.tile([P, S], f32, tag="ob")
                            if st % 2 == 0:
                                nc.scalar.copy(ob[:], ps[:])
                            else:
                                nc.vector.tensor_copy(ob[:], ps[:])
                            nc.sync.dma_start(
                                out_d[on][st * P:(st + 1) * P, :], ob[:])

    nc.compile()
    return nc


def make_in_maps(inputs):
    """Shard + host-prep the full inputs into 8 per-core input maps.

    Everything is packed into one bf16 blob (x, w, wo - in the order the
    kernel's blob16 views expect) and one f32 blob (biases) per core, so
    the per-call relay overhead is paid for 3 buffers instead of 24.
    """
    inp = {k: np.asarray(v) for k, v in inputs.items()}
    # per-batch x block: xq_r, xq_i, xk_r, xk_i, xv_r, xv_i transposed, flat
    x_blocks = []
    for b in range(4):
        parts = [np.ascontiguousarray(inp[n][b].T).astype(bf16).ravel()
                 for n in ("query_real", "query_imag", "key_real", "key_imag",
                           "value_real", "value_imag")]
        x_blocks.append(np.concatenate(parts))
    # per-group w/wo block and f32 bias blob
    w_blocks, b32_blocks = [], []
    for g in range(2):
        rows = slice(g * DL, (g + 1) * DL)
        parts = [np.ascontiguousarray(inp[n][rows].T).astype(bf16).ravel()
                 for n in ("Wq_r", "Wq_i", "Wk_r", "Wk_i", "Wv_r", "Wv_i")]
        parts += [np.ascontiguousarray(inp[n][:, rows].T).astype(bf16).ravel()
                  for n in ("Wo_r", "Wo_i")]
        w_blocks.append(np.concatenate(parts))
        b32 = [np.ascontiguousarray(
                   inp[n][rows].reshape(4, P).T).astype(np.float32).ravel()
               for n in ("bq_r", "bq_i", "bk_r", "bk_i")]
        b32 += [np.ascontiguousarray(np.broadcast_to(
                    inp[n][rows], (P, DL))).astype(np.float32).ravel()
                for n in ("bv_r", "bv_i")]
        b32_blocks.append(np.concatenate(b32))

    in_maps = []
    for c in range(8):
        b, g = c // 2, c % 2
        in_maps.append({
            "blob16": np.concatenate([x_blocks[b], w_blocks[g]]),
            "blob32": b32_blocks[g],
        })
    return in_maps


def combine_outputs(results, inputs):
    bo_r = np.asarray(inputs["bo_r"], np.float32)
    bo_i = np.asarray(inputs["bo_i"], np.float32)
    B = 4
    out_r = np.empty((B, S, E), np.float32)
    out_i = np.empty((B, S, E), np.float32)
    for b in range(B):
        out_r[b] = results[2 * b]["out"][0] + results[2 * b + 1]["out"][0] + bo_r
        out_i[b] = results[2 * b]["out"][1] + results[2 * b + 1]["out"][1] + bo_i
    return out_r, out_i


def get_nc():
    global _NC_CACHE
    if _NC_CACHE is None:
        _NC_CACHE = _build()
    return _NC_CACHE


def kernel(**inputs):
    from concourse.bass_utils import run_bass_kernel_spmd

    nc = get_nc()
    in_maps = make_in_maps(inputs)
    res = run_bass_kernel_spmd(nc, in_maps, list(range(8)))
    return combine_outputs(res.results, inputs)

